# revision 1
# baseline (speedup 1.0000x reference)
"""Trainium2 Bass kernel for nn_Attention_org_10514079941402.

Math reduction: for each sample n (emb[n] is [T=8, D=2048]):
  G[n]      = emb[n] @ emb[n].T                      (8x8 Gram, contracts D)
  scores[h] = Wq[h] @ G[n] @ Wk[h].T / sqrt(T) + bias[h]
  probs     = softmax(instancenorm(scores))
  M[n]      = (1/H) * Wo @ (sum_h probs[h] @ Wv[h])  (8x8)
  out[n]    = M[n] @ emb[n]

Device pass 1 computes G for all samples (the only quadratic-in-emb part),
the tiny 8x8 chain runs on host, device pass 2 applies out = M @ emb via
block-diagonal stationary matrices (16 samples = 128 rows per matmul).
Data parallel over the leading N axis across 8 cores.

Walrus constraint: a PE instruction carries at most ONE sync wait. fp32
matmuls self-load weights (no separate LDWEIGHTS to spread waits over), so
bf16 LDWEIGHTS instructions are inserted as pure wait-carriers: they absorb
the cross-engine data waits, leaving each matmul only its PSUM-bank wait.
The garbage weights they load are irrelevant (fp32 matmuls self-load).
"""

import numpy as np

import concourse.bass as bass
import concourse.mybir as mybir
import concourse.tile as tile
from concourse.bass_utils import run_bass_kernel_spmd

PROFILE = False          # set by test harness; adds NTFF tracing
LAST_EXEC_NS = []        # per-launch HW exec times when PROFILE

N, T, D, H = 2048, 8, 2048, 4
NCORES = 8
NPC = N // NCORES            # 256 samples per core
GRP = 16                     # samples per 128-row group
GROUPS = NPC // GRP          # 16 groups per core
ROWS = NPC * T               # 2048 rows per core
EPS = 1e-5
FP = mybir.dt.float32
FPR = mybir.dt.float32r
BF = mybir.dt.bfloat16
NCHUNK = D // 128            # 16 transpose/gram chunks
NJ = D // 512                # 4 apply matmuls per group


def _carrier(nc, ap64):
    """bf16 LDWEIGHTS reading ap64 (a [128, 64] fp32 slice): absorbs the
    producer's semaphore wait onto a write-free PE instruction."""
    nc.tensor.ldweights(ap64.bitcast(BF))


def _strip_self_waits(nc):
    """Walrus accepts only ONE sync wait per engine instruction.

    1. Tile emits same-engine self-waits for slot releases; on strict-FIFO
       engines (DVE, ACT) program order already guarantees them - drop them.
    2. Any instruction still carrying >=2 waits gets the extras hoisted onto
       single-wait Drain instructions inserted just before it (same engine).
    """
    pref = {"EngineType.DVE": "DVE", "EngineType.ACT": "ACT",
            "EngineType.Activation": "ACT"}
    for blk in nc.m.functions[0].blocks:
        idx = 0
        insts = blk.instructions
        while idx < len(insts):
            inst = insts[idx]
            si = inst.sync_info
            if si is None:
                idx += 1
                continue
            waits = list(si.on_wait)
            if len(waits) < 2:
                idx += 1
                continue
            p = pref.get(str(inst.engine))
            if p is not None:
                keep = [w for w in waits if not w.ant_name.startswith(p)]
                if 1 <= len(keep) < len(waits):
                    waits = keep
            if len(waits) >= 2:
                for k, w in enumerate(waits[:-1]):
                    d = mybir.InstDrain(
                        name=f"{inst.name}_w{k}", ins=[], outs=[],
                        sync_info=mybir.SyncInfo(on_wait=[w], on_update=[]),
                    )
                    d.engine = inst.engine
                    insts.insert(idx, d)
                    idx += 1
                waits = [waits[-1]]
            inst.sync_info = mybir.SyncInfo(
                on_wait=waits, on_update=list(si.on_update)
            )
            idx += 1
    return nc


def _build_gram():
    nc = bass.Bass()
    emb = nc.dram_tensor("emb", [ROWS, D], FP, kind="ExternalInput")
    ident = nc.dram_tensor("ident", [128, 128], FP, kind="ExternalInput")
    gout = nc.dram_tensor("gout", [GROUPS, 128, 128], FP, kind="ExternalOutput")
    embr = emb[:, :].rearrange("(g p) d -> p g d", p=128)   # [128, GROUPS, D]
    with tile.TileContext(nc) as tc:
        with tc.tile_pool(name="const", bufs=1) as cpool, \
             tc.tile_pool(name="eb", bufs=1) as epool, \
             tc.tile_pool(name="et", bufs=2) as etpool, \
             tc.tile_pool(name="gsa", bufs=1) as gspool, \
             tc.tile_pool(name="etq0", bufs=1, space="PSUM") as psq0, \
             tc.tile_pool(name="etq1", bufs=1, space="PSUM") as psq1, \
             tc.tile_pool(name="etq2", bufs=1, space="PSUM") as psq2, \
             tc.tile_pool(name="etq3", bufs=1, space="PSUM") as psq3, \
             tc.tile_pool(name="gp", bufs=4, space="PSUM") as gppool:
            ident_sb = cpool.tile([128, 128], FP, name="ident_sb")
            nc.sync.dma_start(out=ident_sb[:], in_=ident[:, :])
            _carrier(nc, ident_sb[:, 0:64])
            # 4 chunked loads, 4 groups each, into 4 distinct tiles (no slot
            # reuse -> zero waits on load DMAs; <=8 HWDGE DMAs total keeps
            # every DMA on a private semaphore lane).
            GC = GROUPS // 4
            echunks = []
            for q in range(4):
                ec = epool.tile([128, GC, D], FP, name=f"ec{q}", tag=f"ec{q}")
                nc.sync.dma_start(out=ec[:], in_=embr[:, q * GC:(q + 1) * GC, :])
                echunks.append(ec)
            gs_all = gspool.tile([128, GROUPS, 128], FP, name="gs_all")

            def emit_transposes(g):
                e = echunks[g // GC][:, g % GC, :]
                _carrier(nc, e[:, 0:64])
                ets = etpool.tile([128, D], FP, name="ets", tag="ets")
                QC = NCHUNK // 4
                QW = D // 4
                for qi, pool in enumerate((psq0, psq1, psq2, psq3)):
                    etp = pool.tile([128, QW], FP, name=f"etq{qi}",
                                    tag=f"etq{qi}")
                    for ci in range(QC):
                        c = qi * QC + ci
                        nc.tensor.transpose(
                            out=etp[:, ci * 128:(ci + 1) * 128],
                            in_=e[:, c * 128:(c + 1) * 128],
                            identity=ident_sb[:],
                        )
                    dst = ets[:, qi * QW:(qi + 1) * QW]
                    if qi % 2 == 0:
                        nc.vector.tensor_copy(dst, etp[:])
                    else:
                        nc.scalar.copy(dst, etp[:])
                return ets

            def emit_grams(g, ets):
                _carrier(nc, ets[:, 2 * (D // 4):2 * (D // 4) + 64])
                _carrier(nc, ets[:, 3 * (D // 4):3 * (D // 4) + 64])
                gp = gppool.tile([128, 128], FP, name="gp", tag="gp")
                for c in range(NCHUNK):
                    nc.tensor.matmul(
                        gp[:],
                        ets[:, c * 128:(c + 1) * 128],
                        ets[:, c * 128:(c + 1) * 128],
                        start=(c == 0),
                        stop=(c == NCHUNK - 1),
                    )
                nc.vector.tensor_copy(gs_all[:, g, :], gp[:])

            # One-group software pipeline: group g+1's transposes are emitted
            # before group g's gram matmuls, so PE never stalls waiting for
            # the PSUM->SBUF copies of the group it is about to contract.
            pend = (0, emit_transposes(0))
            for g in range(1, GROUPS):
                ets = emit_transposes(g)
                emit_grams(*pend)
                pend = (g, ets)
            emit_grams(*pend)
            nc.sync.dma_start(
                out=gout[:, :, :].rearrange("g p c -> p g c"), in_=gs_all[:]
            )
    return _strip_self_waits(nc)


def _build_apply():
    nc = bass.Bass()
    emb = nc.dram_tensor("emb", [ROWS, D], FP, kind="ExternalInput")
    bd = nc.dram_tensor("bd", [GROUPS, 128, 128], FP, kind="ExternalInput")
    outp = nc.dram_tensor("outp", [ROWS, D], FP, kind="ExternalOutput")
    embr = emb[:, :].rearrange("(g p) d -> p g d", p=128)
    outr = outp[:, :].rearrange("(g p) d -> p g d", p=128)
    with tile.TileContext(nc) as tc:
        with tc.tile_pool(name="bda", bufs=1) as bdapool, \
             tc.tile_pool(name="eb", bufs=1) as epool, \
             tc.tile_pool(name="os2", bufs=4) as ospool, \
             tc.tile_pool(name="op", bufs=2, space="PSUM") as oppool:
            bd_sb = bdapool.tile([128, GROUPS, 128], FP, name="bd_sb")
            nc.sync.dma_start(
                out=bd_sb[:], in_=bd[:, :, :].rearrange("g p c -> p g c")
            )
            _carrier(nc, bd_sb[:, 0, 0:64])
            GC = GROUPS // 4
            echunks = []
            for q in range(4):
                ec = epool.tile([128, GC, D], FP, name=f"ec{q}", tag=f"ec{q}")
                nc.sync.dma_start(out=ec[:], in_=embr[:, q * GC:(q + 1) * GC, :])
                echunks.append(ec)
            os_hist = []
            os2 = None
            for g in range(GROUPS):
                e = echunks[g // GC][:, g % GC, :]
                _carrier(nc, e[:, 0:64])
                os2 = ospool.tile([128, D], FP, name="os2", tag="os2")
                os_hist.append(os2)
                if g >= 2:
                    prev = os_hist[g - 2]
                    # op slot reuse: absorb both copy-engine dependencies.
                    _carrier(nc, prev[:, 0:64])
                    _carrier(nc, prev[:, D // 2:D // 2 + 64])
                for hhalf in range(2):
                    oph = oppool.tile([128, D // 2], FP, name="oph",
                                      tag=f"oph{hhalf}")
                    for jj in range(2):
                        j = hhalf * 2 + jj
                        nc.tensor.matmul(
                            oph[:, jj * 512:(jj + 1) * 512],
                            bd_sb[:, g, :],
                            e[:, j * 512:(j + 1) * 512],
                            start=True,
                            stop=True,
                        )
                    dst = os2[:, hhalf * (D // 2):(hhalf + 1) * (D // 2)]
                    if hhalf == 0:
                        nc.vector.tensor_copy(dst, oph[:])
                    else:
                        nc.scalar.copy(dst, oph[:])
                nc.sync.dma_start(out=outr[:, g, :], in_=os2[:])
    return _strip_self_waits(nc)


def _host_small_math(Gn, Wq, Wk, Wv, Wo, rel_table):
    """Gn [N,T,T] -> M [N,T,T] with out[n] = M[n] @ emb[n]."""
    scale = np.float32(1.0 / np.sqrt(T))
    scores = np.einsum("hta,nab,hsb->nhts", Wq, Gn, Wk) * scale
    idx = np.arange(T)[:, None] - np.arange(T)[None, :] + T - 1
    bias = rel_table[idx]                      # [T,T,H]
    scores = scores + bias.transpose(2, 0, 1)[None]
    mu = scores.mean(axis=(2, 3), keepdims=True)
    var = scores.var(axis=(2, 3), keepdims=True)
    scores = (scores - mu) / np.sqrt(var + EPS)
    scores = scores - scores.max(axis=-1, keepdims=True)
    ex = np.exp(scores)
    probs = ex / ex.sum(axis=-1, keepdims=True)
    A = np.einsum("nhts,hsu->ntu", probs, Wv) / np.float32(H)
    M = np.einsum("tu,nus->nts", Wo, A)
    return M.astype(np.float32)


def kernel(emb, Wq, Wk, Wv, Wo, rel_table):
    emb = np.ascontiguousarray(emb, dtype=np.float32)
    Wq = np.asarray(Wq, np.float32)
    Wk = np.asarray(Wk, np.float32)
    Wv = np.asarray(Wv, np.float32)
    Wo = np.asarray(Wo, np.float32)
    rel_table = np.asarray(rel_table, np.float32)

    embc = emb.reshape(NCORES, ROWS, D)
    ident = np.eye(128, dtype=np.float32)
    core_ids = list(range(NCORES))

    del LAST_EXEC_NS[:]
    nc1 = _build_gram()
    r1 = run_bass_kernel_spmd(
        nc1, [{"emb": embc[i], "ident": ident} for i in range(NCORES)], core_ids,
        trace=PROFILE,
    )
    if PROFILE:
        LAST_EXEC_NS.append(r1.exec_time_ns)
    G = np.stack([r1.results[i]["gout"] for i in range(NCORES)])
    # [C, GROUPS, 128, 128] -> diagonal 8x8 blocks -> [N, T, T]
    Gb = G.reshape(NCORES, GROUPS, GRP, T, GRP, T)
    Gn = np.einsum("cgbtbs->cgbts", Gb).reshape(N, T, T)

    M = _host_small_math(Gn, Wq, Wk, Wv, Wo, rel_table)

    Mn = M.reshape(NCORES, GROUPS, GRP, T, T)
    bd = np.zeros((NCORES, GROUPS, GRP, T, GRP, T), np.float32)
    for b in range(GRP):
        # BD[(b,s),(b,t)] = M[b][t,s]
        bd[:, :, b, :, b, :] = Mn[:, :, b].swapaxes(-1, -2)
    bd = bd.reshape(NCORES, GROUPS, 128, 128)

    nc2 = _build_apply()
    r2 = run_bass_kernel_spmd(
        nc2,
        [{"emb": embc[i], "bd": bd[i]} for i in range(NCORES)],
        core_ids,
        trace=PROFILE,
    )
    if PROFILE:
        LAST_EXEC_NS.append(r2.exec_time_ns)
    out = np.stack([r2.results[i]["outp"] for i in range(NCORES)])
    return out.reshape(N, T, D)

